# revision 5
# baseline (speedup 1.0000x reference)
"""Trainium2 Bass kernel for nn_FeatureContraction.

Computes out[b,c,w,x,v] = sum_i x[b,c,w,x,v,i] * node_attributes[b,c,i]
with B=C=128, X=3, Y=16 (wxv = 3*16*16 = 768, i = 16).

Strategy (8 NeuronCores, data-parallel over b, bandwidth-asymmetric):
  - the 8 NCs on this chip have measurably different sustained HBM
    read bandwidth under full load: odd NCs ~425 GB/s, even NCs
    ~330 GB/s (stable arbitration asymmetry, same for SWDGE/HWDGE).
    SPMD model index preserves NC parity, so the shard is asymmetric:
    even models process 14 b-slices, odd models 18 (14 unconditional
    + 4 inside a `tc.If(partition_id % 2 == 1)` block).
  - SBUF layout: partitions = c (128), free = contiguous (wxv, i)
    -> DMA reads 48 KiB contiguous per partition (full HBM rate).
    The load casts f32 -> bf16 in the DMA datapath (SWDGE cast).
  - multiply: tmp[c, w, i] = x[c, w, i] * na[c, i] with a step-0
    broadcast AP on na.
  - reduce over i, split by w to balance engines:
      w < RED_SPLIT: DVE grouped tensor_reduce (innermost axis)
      w >= RED_SPLIT: 16 identity-weight PE matmuls accumulating the
      strided i-slices into PSUM, then ACT copies PSUM->SBUF.
  - output stored as bf16 (tolerance is 2e-2; halves the HBM write
    traffic), cast back to f32 on the host after the gather.
  - first b-slice loaded in 4 quarter-chunks: the SWDGE Q7 descriptor
    emission for a full 6 MiB slice delays the first HBM byte by ~8 us;
    small first transfers start the stream almost immediately.
  - last b-slice (after the conditional block, common to both paths)
    also in 4 quarters ordered PE-part first, DVE-part last, so the
    post-DMA pipeline tail is short.
"""

import sys

for _p in ("/opt/trn_rl_repo",):
    if _p not in sys.path:
        sys.path.append(_p)

import numpy as np

import concourse.bass as bass
import concourse.mybir as mybir
import concourse.tile as tile
from concourse import bacc
from concourse.bass_utils import run_bass_kernel_spmd

# Problem dims (hardcoded per spec)
B, C, X, Y = 128, 128, 3, 16
WXV = X * Y * Y          # 768
I = Y                    # 16 (contraction axis)
N_CORES = 8
B_MAIN = 14              # unconditional b-slices per core
B_EXTRA = 4              # extra b-slices on odd (fast) models
B_TOT = B_MAIN + B_EXTRA
# per-core slice counts by model parity: 4*14 + 4*18 = 128 = B
SIZES = [B_MAIN + B_EXTRA * (k % 2) for k in range(N_CORES)]
OFFS = np.cumsum([0] + SIZES).tolist()
assert OFFS[-1] == B

RED_SPLIT = 336          # DVE reduces w < RED_SPLIT, PE reduces the rest
Q = WXV // 4             # 192-wide quarter chunks for the edge slices

F32 = mybir.dt.float32
BF16 = mybir.dt.bfloat16

_COMPILED = None


def _build():
    nc = bacc.Bacc("TRN2", target_bir_lowering=False, debug=False,
                   num_devices=N_CORES)

    x_d = nc.dram_tensor("x", [B_MAIN, C, WXV, I], F32, kind="ExternalInput")
    xe_d = nc.dram_tensor("xe", [B_EXTRA, C, WXV, I], F32,
                          kind="ExternalInput")
    na_d = nc.dram_tensor("naT", [C, B_TOT, I], F32, kind="ExternalInput")
    eye_d = nc.dram_tensor("eye", [C, C], F32, kind="ExternalInput")
    out_d = nc.dram_tensor("out", [B_MAIN, C, WXV], BF16,
                           kind="ExternalOutput")
    oute_d = nc.dram_tensor("oute", [B_EXTRA, C, WXV], BF16,
                            kind="ExternalOutput")

    WA = RED_SPLIT
    WB = WXV - RED_SPLIT

    with tile.TileContext(nc) as tc:
        nc.cache_partition_id()
        pid = nc.partition_id()
        with (
            tc.tile_pool(name="const", bufs=1) as constp,
            tc.tile_pool(name="xp", bufs=3) as xp,
            tc.tile_pool(name="xq", bufs=4) as xqp,
            tc.tile_pool(name="tmpp", bufs=3) as tmpp,
            tc.tile_pool(name="tmpq", bufs=4) as tmpqp,
            tc.tile_pool(name="outp", bufs=3) as outp,
            tc.tile_pool(name="psp", bufs=4, space="PSUM") as psp,
        ):
            eye = constp.tile([C, C], BF16)
            na_sb = constp.tile([C, B_TOT, I], BF16)
            eye_f = constp.tile([C, C], F32)
            na_f = constp.tile([C, B_TOT, I], F32)

            def quarter(src, na_row, q, use_pe, ot):
                """Load + process quarter q (w in [q*Q,(q+1)*Q)) of a slice."""
                xt = xqp.tile([C, Q, I], BF16, tag="xq")
                nc.gpsimd.dma_start(xt[:], src[:, q * Q:(q + 1) * Q, :])
                nab = na_sb[:, na_row, :][:, None, :]
                tq = tmpqp.tile([C, Q, I], BF16, tag="tmpq")
                nc.vector.tensor_mul(tq[:], xt[:], nab.broadcast_to([C, Q, I]))
                oq = ot[:, q * Q:(q + 1) * Q]
                if use_pe:
                    ps = psp.tile([C, Q], F32, tag="psq")
                    for i in range(I):
                        nc.tensor.matmul(ps[:], eye[:], tq[:, :, i],
                                         start=(i == 0), stop=(i == I - 1))
                    nc.scalar.copy(oq, ps[:])
                else:
                    with nc.allow_low_precision(reason="bf16 out, tol 2e-2"):
                        nc.vector.tensor_reduce(oq, tq[:], mybir.AxisListType.X,
                                                mybir.AluOpType.add)

            def full_slice(src, na_row, odst):
                """Load + process one full slice; store to odst ([C, WXV])."""
                xt = xp.tile([C, WXV, I], BF16, tag="x")
                nc.gpsimd.dma_start(xt[:], src)
                ot = outp.tile([C, WXV], BF16, tag="out")
                nab = na_sb[:, na_row, :][:, None, :]
                xt_b, xt_a = xt[:, RED_SPLIT:, :], xt[:, :RED_SPLIT, :]
                # B half: mult then 16 PE identity matmuls (psum accumulate)
                tb = tmpp.tile([C, WB, I], BF16, tag="tmpb")
                nc.vector.tensor_mul(tb[:], xt_b,
                                     nab.broadcast_to([C, WB, I]))
                ps = psp.tile([C, WB], F32, tag="ps")
                for i in range(I):
                    nc.tensor.matmul(ps[:], eye[:], tb[:, :, i],
                                     start=(i == 0), stop=(i == I - 1))
                # A half: mult then DVE grouped reduce
                ta = tmpp.tile([C, WA, I], BF16, tag="tmpa")
                nc.vector.tensor_mul(ta[:], xt_a,
                                     nab.broadcast_to([C, WA, I]))
                nc.scalar.copy(ot[:, RED_SPLIT:], ps[:])
                with nc.allow_low_precision(reason="bf16 out, tol 2e-2"):
                    nc.vector.tensor_reduce(ot[:, :RED_SPLIT], ta[:],
                                            mybir.AxisListType.X,
                                            mybir.AluOpType.add)
                nc.scalar.dma_start(odst, ot[:])

            # --- slice 0: quartered for a fast DMA stream start ---
            ot0 = outp.tile([C, WXV], BF16, tag="out")
            first = xqp.tile([C, Q, I], BF16, tag="xq")
            nc.gpsimd.dma_start(first[:], x_d[0, :, 0:Q, :])
            nc.sync.dma_start(eye_f[:], eye_d[:])
            nc.sync.dma_start(na_f[:], na_d[:])
            nc.vector.tensor_copy(eye[:], eye_f[:])
            nc.vector.tensor_copy(na_sb[:], na_f[:])
            nab0 = na_sb[:, 0, :][:, None, :]
            tq0 = tmpqp.tile([C, Q, I], BF16, tag="tmpq")
            nc.vector.tensor_mul(tq0[:], first[:], nab0.broadcast_to([C, Q, I]))
            with nc.allow_low_precision(reason="bf16 out, tol 2e-2"):
                nc.vector.tensor_reduce(ot0[:, 0:Q], tq0[:],
                                        mybir.AxisListType.X,
                                        mybir.AluOpType.add)
            quarter(x_d[0], 0, 1, False, ot0)
            quarter(x_d[0], 0, 2, True, ot0)
            quarter(x_d[0], 0, 3, True, ot0)
            nc.scalar.dma_start(out_d[0], ot0[:])

            # --- middle slices: full 6 MiB loads ---
            for b in range(1, B_MAIN - 1):
                full_slice(x_d[b], b, out_d[b])

            # --- extra slices: odd (fast) models only ---
            with tc.If(pid % 2 == 1):
                for e in range(B_EXTRA):
                    full_slice(xe_d[e], B_MAIN + e, oute_d[e])

            # --- last slice: quartered, PE parts first, DVE part last ---
            b = B_MAIN - 1
            otl = outp.tile([C, WXV], BF16, tag="out")
            quarter(x_d[b], b, 2, True, otl)
            quarter(x_d[b], b, 3, True, otl)
            nc.scalar.dma_start(out_d[b, :, 2 * Q:], otl[:, 2 * Q:])
            quarter(x_d[b], b, 0, False, otl)
            quarter(x_d[b], b, 1, False, otl)
            nc.scalar.dma_start(out_d[b, :, :2 * Q], otl[:, :2 * Q])

    nc.compile()
    return nc


def _get_compiled():
    global _COMPILED
    if _COMPILED is None:
        _COMPILED = _build()
    return _COMPILED


def _make_in_maps(inputs: dict):
    x = np.ascontiguousarray(np.asarray(inputs["x"], dtype=np.float32))
    na = np.asarray(inputs["node_attributes"], dtype=np.float32)

    x_sh = x.reshape(B, C, WXV, I)
    naT = np.ascontiguousarray(na.transpose(1, 0, 2))  # [C, B, I]
    eye = np.eye(C, dtype=np.float32)
    xe_zero = np.zeros((B_EXTRA, C, WXV, I), np.float32)

    in_maps = []
    for k in range(N_CORES):
        b0, n = OFFS[k], SIZES[k]
        na_k = np.zeros((C, B_TOT, I), np.float32)
        na_k[:, :n, :] = naT[:, b0:b0 + n, :]
        in_maps.append(
            {
                "x": x_sh[b0:b0 + B_MAIN],
                "xe": (np.ascontiguousarray(x_sh[b0 + B_MAIN:b0 + n])
                       if n > B_MAIN else xe_zero),
                "naT": na_k,
                "eye": eye,
            }
        )
    return in_maps


def _gather(results) -> np.ndarray:
    parts = []
    for k, r in enumerate(results):
        parts.append(np.asarray(r["out"]))
        if SIZES[k] > B_MAIN:
            parts.append(np.asarray(r["oute"]))
    out = np.concatenate(parts, axis=0)
    return out.astype(np.float32).reshape(B, C, X, Y, Y)


def _run(inputs: dict, trace: bool = False, trace_cores=None):
    in_maps = _make_in_maps(inputs)
    nc = _get_compiled()
    res = run_bass_kernel_spmd(
        nc,
        in_maps,
        core_ids=list(range(N_CORES)),
        trace=trace,
        trace_cores=trace_cores,
    )
    return _gather(res.results), res


def kernel(**inputs) -> np.ndarray:
    out, _ = _run(inputs, trace=False)
    return out
